# revision 1
# baseline (speedup 1.0000x reference)
"""Causal self-attention (B=2, S=2048, D=1024, H=16) on 8 TRN2 NeuronCores.

Sharding: strided-query data parallel. Core c handles batch c//4 and query
rows {4i + c%4 : i in 0..511} (stride-4 interleave). With this layout the
causal structure is identical on every core: for kv chunk j (128 keys),
exactly the local query columns [32j, 512) attend to it, and the partial
(diagonal) block is always the first 32 of those columns with a mask that is
independent of j. So attention does only the causally-required work — no
fully-masked tiles — with a uniform SPMD program.

All matmuls run in bf16 (1 cycle/row at any free size). K and V live
entirely in SBUF (no DRAM round trip). Each core computes the full K/V
projection for its batch (redundant 4x, but avoids collectives).

Per-core dataflow:
  A: constants, bias broadcasts (ones-matmul)
  B: Q^T = bf16(scale*(Wq^T xq + bq)) [128,512] x8; K^T -> SBUF [8][128,2048]
     bf16 (bias dropped: softmax-invariant); V (+ones col) -> SBUF
     [16][128,16*65] bf16
  C: per head h, kv chunks packed into PSUM banks in pairs
     (0),(1,15),(2,14),...,(8): S^T = K^T.T Q^T over cols [32j,512),
     diagonal mask add (DVE, [128,32]), exp (ACT) -> P bf16;
     O^T[65,512] += [V|1]_j.T P_j (row 64 = softmax denominator);
     normalize via reciprocal + ones-broadcast matmul
  D: out = y @ Wp + bp per [128,512] tile, DMA out
"""

import numpy as np
import ml_dtypes

import concourse.bass as bass
import concourse.mybir as mybir
import concourse.tile as tile
from concourse import bacc
from concourse.bass_utils import run_bass_kernel_spmd

F32 = mybir.dt.float32
F32R = mybir.dt.float32r
BF16 = mybir.dt.bfloat16
AF = mybir.ActivationFunctionType
ALU = mybir.AluOpType

B, S, D, H, HD = 2, 2048, 1024, 16, 64
QL = 512          # query rows per core
NKC = D // 128    # 8 model-dim chunks
NHP = H // 2      # 8 head pairs
KVC = S // 128    # 16 kv chunks
NEG = -1.0e9
SCALE = 1.0 / np.sqrt(HD)

# kv-chunk processing order: chunk pairs that share one PSUM bank
# (widths 512-32j; pair sums <= 512). Chunk 0 must come first (O start flag).
GROUPS = [(0,), (1, 15), (2, 14), (3, 13), (4, 12), (5, 11), (6, 10), (7, 9),
          (8,)]

_CACHED = {}


def build_nc(reps=1):
    nc = bacc.Bacc("TRN2", target_bir_lowering=False, debug=False)

    xt = nc.dram_tensor("xt", [D, S], BF16, kind="ExternalInput").ap()
    xqt = nc.dram_tensor("xqt", [D, QL], BF16, kind="ExternalInput").ap()
    wa = nc.dram_tensor("wa", [D, 3 * D], BF16, kind="ExternalInput").ap()
    baq = nc.dram_tensor("baq", [D], F32, kind="ExternalInput").ap()
    bav = nc.dram_tensor("bav", [1, D], F32R, kind="ExternalInput").ap()
    wp = nc.dram_tensor("wp", [D, D], BF16, kind="ExternalInput").ap()
    bp = nc.dram_tensor("bp", [1, D], F32R, kind="ExternalInput").ap()
    mask = nc.dram_tensor("mask", [128, 32], F32, kind="ExternalInput").ap()
    ones1 = nc.dram_tensor("ones1", [1, 128], F32R, kind="ExternalInput").ap()
    out = nc.dram_tensor("out", [QL, D], F32, kind="ExternalOutput").ap()

    with tile.TileContext(nc) as tc:
        for _ in range(reps):
            _body(nc, tc, xt, xqt, wa, baq, bav, wp, bp, mask, ones1, out)
    nc.compile()
    return nc


def _body(nc, tc, xt, xqt, wa, baq, bav, wp, bp, mask, ones1, out):
    with (
        tc.tile_pool(name="const", bufs=1) as const_p,
        tc.tile_pool(name="qt", bufs=1) as qt_p,
        tc.tile_pool(name="kt", bufs=1) as kt_p,
        tc.tile_pool(name="vs", bufs=1) as vs_p,
        tc.tile_pool(name="wkp", bufs=1) as wk_p,
        tc.tile_pool(name="xtp", bufs=1) as xt_p,
    ):
        # ---------- phase A: constants ----------
        ones_s = const_p.tile([1, 128], F32R)
        nc.sync.dma_start(ones_s[:], ones1[:])
        bav_s = const_p.tile([1, D], F32R)
        nc.sync.dma_start(bav_s[:], bav[:])
        bp_s = const_p.tile([1, D], F32R)
        nc.sync.dma_start(bp_s[:], bp[:])
        mask_s = const_p.tile([128, 32], F32)
        nc.sync.dma_start(mask_s[:], mask[:])
        baq_s = [const_p.tile([128, 1], F32, tag=f"baq{i}", name=f"baq{i}")
                 for i in range(NHP)]
        for hp in range(NHP):
            nc.sync.dma_start(
                baq_s[hp][:],
                baq[hp * 128:(hp + 1) * 128].rearrange("(p o) -> p o", o=1),
            )
        bav_bc = const_p.tile([128, D], F32)
        bp_bc = const_p.tile([128, D], F32)
        with tc.tile_pool(name="psumA", bufs=2, space="PSUM") as psumA:
            for n2 in range(2):
                t = psumA.tile([128, 512], F32, tag="ps", name=f"bcv{n2}")
                nc.tensor.matmul(t[:], ones_s[:],
                                 bav_s[:, n2 * 512:(n2 + 1) * 512],
                                 start=True, stop=True)
                nc.vector.tensor_copy(bav_bc[:, n2 * 512:(n2 + 1) * 512], t[:])
                t2 = psumA.tile([128, 512], F32, tag="ps", name=f"bcp{n2}")
                nc.tensor.matmul(t2[:], ones_s[:],
                                 bp_s[:, n2 * 512:(n2 + 1) * 512],
                                 start=True, stop=True)
                nc.vector.tensor_copy(bp_bc[:, n2 * 512:(n2 + 1) * 512],
                                      t2[:])

        qt_s = [qt_p.tile([128, QL], BF16, tag=f"qt{hp}", name=f"qt{hp}")
                for hp in range(NHP)]
        kt_s = [kt_p.tile([128, S], BF16, tag=f"kt{hp}", name=f"kt{hp}")
                for hp in range(NHP)]
        v_s = [vs_p.tile([128, H * 65], BF16, tag=f"v{j}", name=f"v{j}")
               for j in range(KVC)]

        # ---------- phase B: V, Q, and K(hp=0) projection ----------
        # wk/xt stay resident: K for hp>=1 is emitted inside the attention
        # loop (just-in-time, two [128,512] groups per head) so the PE chews
        # it during the exp-bound stretches instead of idling.
        copies = [
            lambda o, i: nc.vector.tensor_copy(o, i),
            lambda o, i: nc.scalar.copy(o, i),
        ]
        wk_s = [wk_p.tile([128, D], BF16, tag=f"wk{kc}", name=f"wk{kc}")
                for kc in range(NKC)]
        xt_s = [xt_p.tile([128, S], BF16, tag=f"xt{kc}", name=f"xt{kc}")
                for kc in range(NKC)]
        with (
            tc.tile_pool(name="wqv", bufs=1) as wqv_p,
            tc.tile_pool(name="xqp", bufs=1) as xq_p,
            tc.tile_pool(name="psumB", bufs=4, space="PSUM") as psum,
        ):
            wq_s = [wqv_p.tile([128, D], BF16, tag=f"wq{kc}", name=f"wq{kc}")
                    for kc in range(NKC)]
            wv_s = [wqv_p.tile([128, D], BF16, tag=f"wv{kc}", name=f"wv{kc}")
                    for kc in range(NKC)]
            xq_s = [xq_p.tile([128, QL], BF16, tag=f"xq{kc}", name=f"xq{kc}")
                    for kc in range(NKC)]
            # V-path inputs first (V computed first), then Q, then K weights
            for kc in range(NKC):
                nc.sync.dma_start(wv_s[kc][:],
                                  wa[kc * 128:(kc + 1) * 128, 2 * D:3 * D])
                nc.sync.dma_start(xt_s[kc][:], xt[kc * 128:(kc + 1) * 128, :])
            for kc in range(NKC):
                nc.sync.dma_start(wq_s[kc][:],
                                  wa[kc * 128:(kc + 1) * 128, 0:D])
                nc.sync.dma_start(xq_s[kc][:], xqt[kc * 128:(kc + 1) * 128, :])
            for kc in range(NKC):
                nc.sync.dma_start(wk_s[kc][:],
                                  wa[kc * 128:(kc + 1) * 128, D:2 * D])

            # V (natural layout, + ones column at slot 64 of each head)
            for t16 in range(KVC):
                v3 = v_s[t16][:].rearrange("p (h e) -> p h e", e=65)
                for nh in range(2):
                    ps = psum.tile([128, 512], F32, tag="ps", name=f"v{t16}_{nh}")
                    for kc in range(NKC):
                        nc.tensor.matmul(
                            ps[:],
                            xt_s[kc][:, t16 * 128:(t16 + 1) * 128],
                            wv_s[kc][:, nh * 512:(nh + 1) * 512],
                            start=(kc == 0), stop=(kc == NKC - 1),
                        )
                    nc.vector.tensor_tensor(
                        v3[:, nh * 8:(nh + 1) * 8, 0:64],
                        ps[:].rearrange("p (h e) -> p h e", e=64),
                        bav_bc[:, nh * 512:(nh + 1) * 512].rearrange(
                            "p (h e) -> p h e", e=64),
                        ALU.add,
                    )
                nc.gpsimd.memset(v3[:, :, 64:65], 1.0)

            # Q^T (scaled, biased)
            for hp in range(NHP):
                ps = psum.tile([128, QL], F32, tag="ps", name=f"q{hp}")
                for kc in range(NKC):
                    nc.tensor.matmul(
                        ps[:], wq_s[kc][:, hp * 128:(hp + 1) * 128], xq_s[kc][:],
                        start=(kc == 0), stop=(kc == NKC - 1),
                    )
                nc.vector.tensor_scalar(
                    qt_s[hp][:], ps[:], SCALE, baq_s[hp][:], ALU.mult, ALU.add
                )

            # K^T for hp=0 only (no bias: per-query shift is softmax-invariant)
            for t4 in range(4):
                ps = psum.tile([128, 512], F32, tag="ps", name=f"k0_{t4}")
                for kc in range(NKC):
                    nc.tensor.matmul(
                        ps[:],
                        wk_s[kc][:, 0:128],
                        xt_s[kc][:, t4 * 512:(t4 + 1) * 512],
                        start=(kc == 0), stop=(kc == NKC - 1),
                    )
                copies[t4 % 2](kt_s[0][:, t4 * 512:(t4 + 1) * 512], ps[:])

        # ---------- phases C+D ----------
        with (
            tc.tile_pool(name="yt", bufs=1) as yt_p,
            tc.tile_pool(name="wpp", bufs=1) as wp_p,
        ):
            yt_s = [yt_p.tile([128, QL], F32, tag=f"yt{hp}", name=f"yt{hp}")
                    for hp in range(NHP)]
            ytr_s = [yt_p.tile([128, QL], BF16, tag=f"ytr{hp}", name=f"ytr{hp}")
                     for hp in range(NHP)]
            # preload Wp during attention
            wp_s = [wp_p.tile([128, D], BF16, tag=f"wp{kc}", name=f"wps{kc}")
                    for kc in range(NKC)]
            for kc in range(NKC):
                nc.sync.dma_start(wp_s[kc][:], wp[kc * 128:(kc + 1) * 128, :])

            with (
                tc.tile_pool(name="pt", bufs=4) as pt_p,
                tc.tile_pool(name="den", bufs=1) as den_p,
                tc.tile_pool(name="psumW", bufs=5, space="PSUM") as psumW,
                tc.tile_pool(name="psumR", bufs=1, space="PSUM") as psumR,
                tc.tile_pool(name="opsum", bufs=2, space="PSUM") as opsum,
            ):
                def emit_norm(h):
                    # PE part of head h's normalization, deferred so it never
                    # stalls the S/O matmul stream (recip has finished by now)
                    hp, hh = h // 2, h % 2
                    rp = psumR.tile([128, QL], F32, tag="rp", name=f"rp{h}")
                    nc.tensor.matmul(rp[0:64, :], ones_s[:, 0:64],
                                     rec_by_h.pop(h)[:], start=True, stop=True)
                    nc.vector.tensor_tensor(
                        ytr_s[hp][hh * 64:(hh + 1) * 64, :],
                        yt_s[hp][hh * 64:(hh + 1) * 64, :],
                        rp[0:64, :], ALU.mult,
                    )

                # K projection for hp>=1, emitted just-in-time inside the
                # head loop (K(hp+1) during heads 2hp and 2hp+1)
                k_pending = [(hp2, t4) for hp2 in range(1, NHP)
                             for t4 in range(4)]

                def emit_k_group():
                    if not k_pending:
                        return
                    hp2, t4 = k_pending.pop(0)
                    ps = psumW.tile([128, 512], F32, tag="sw",
                                    name=f"k{hp2}_{t4}")
                    for kc in range(NKC):
                        nc.tensor.matmul(
                            ps[:],
                            wk_s[kc][:, hp2 * 128:(hp2 + 1) * 128],
                            xt_s[kc][:, t4 * 512:(t4 + 1) * 512],
                            start=(kc == 0), stop=(kc == NKC - 1),
                        )
                    copies[(hp2 * 4 + t4) % 2](
                        kt_s[hp2][:, t4 * 512:(t4 + 1) * 512], ps[:])

                rec_by_h = {}
                for h in range(H):
                    hp, hh = h // 2, h % 2
                    op = opsum.tile([65, QL], F32, tag="o", name=f"op{h}")
                    o_idx = 0
                    # software pipeline: S/mask/exp of pair k+1 interleaves
                    # with O of pair k
                    staged = None  # (pt tile, [(j, off, fw)...])
                    for pi, pair in enumerate(GROUPS + [None]):
                        if pair is not None:
                            bank = psumW.tile([128, 512], F32, tag="sw",
                                              name=f"s{h}_{pair[0]}")
                            segs = []
                            off = 0
                            for j in pair:
                                fw = 512 - 32 * j
                                nc.tensor.matmul(
                                    bank[:, off:off + fw],
                                    kt_s[hp][hh * 64:(hh + 1) * 64,
                                             j * 128:(j + 1) * 128],
                                    qt_s[hp][hh * 64:(hh + 1) * 64, 32 * j:512],
                                    start=True, stop=True,
                                )
                                segs.append((j, off, fw))
                                off += fw
                            for j, o, fw in segs:
                                nc.vector.tensor_tensor(
                                    bank[:, o:o + 32], bank[:, o:o + 32],
                                    mask_s[:], ALU.add,
                                )
                            pt = pt_p.tile([128, 512], BF16, tag="p",
                                           name=f"pt{h}_{pair[0]}")
                            nc.scalar.activation(pt[:, 0:off], bank[:, 0:off],
                                                 AF.Exp)
                        if pi == 2 and h > 0:
                            emit_norm(h - 1)
                        if pi in (3, 6):
                            emit_k_group()
                        if staged is not None:
                            spt, ssegs = staged
                            for j, o, fw in ssegs:
                                nc.tensor.matmul(
                                    op[:, 32 * j:512],
                                    v_s[j][:, h * 65:(h + 1) * 65],
                                    spt[:, o:o + fw],
                                    start=(o_idx == 0), stop=(o_idx == KVC - 1),
                                )
                                o_idx += 1
                        staged = (pt, segs) if pair is not None else None

                    den_h = den_p.tile([1, QL], F32, tag="den", name=f"den{h}",
                                       bufs=3)
                    nc.vector.tensor_copy(den_h[:], op[64:65, :])
                    nc.scalar.copy(yt_s[hp][hh * 64:(hh + 1) * 64, :],
                                   op[0:64, :])
                    rec_h = den_p.tile([1, QL], F32R, tag="rec",
                                       name=f"rec{h}", bufs=3)
                    rec_by_h[h] = rec_h
                    with nc.allow_low_precision(reason="f32r denominators"):
                        nc.vector.reciprocal(rec_h[:], den_h[:])
                emit_norm(H - 1)

            # ---------- phase D: out projection ----------
            with (
                tc.tile_pool(name="outp", bufs=3) as out_p,
                tc.tile_pool(name="psumD", bufs=2, space="PSUM") as psumD,
            ):
                for t4 in range(4):
                    for n2 in range(2):
                        ps = psumD.tile([128, 512], F32, tag="ps",
                                        name=f"o{t4}_{n2}")
                        for hp in range(NHP):
                            nc.tensor.matmul(
                                ps[:],
                                ytr_s[hp][:, t4 * 128:(t4 + 1) * 128],
                                wp_s[hp][:, n2 * 512:(n2 + 1) * 512],
                                start=(hp == 0), stop=(hp == NHP - 1),
                            )
                        ot = out_p.tile([128, 512], F32, tag="ot",
                                        name=f"ot{t4}_{n2}")
                        nc.vector.tensor_tensor(
                            ot[:], ps[:], bp_bc[:, n2 * 512:(n2 + 1) * 512],
                            ALU.add,
                        )
                        nc.sync.dma_start(
                            out[t4 * 128:(t4 + 1) * 128,
                                n2 * 512:(n2 + 1) * 512],
                            ot[:],
                        )


def _host_inputs(x, w_attn, b_attn, w_proj, b_proj):
    bf = ml_dtypes.bfloat16
    wa_bf = np.ascontiguousarray(w_attn.astype(np.float32)).astype(bf)
    wp_bf = np.ascontiguousarray(w_proj.astype(np.float32)).astype(bf)
    baq = (SCALE * b_attn[:D]).astype(np.float32)
    bav1 = b_attn[2 * D:3 * D].astype(np.float32).reshape(1, D)
    bp1 = b_proj.astype(np.float32).reshape(1, D)
    ones1 = np.ones((1, 128), np.float32)

    in_maps = []
    for c in range(8):
        b, p = c // 4, c % 4
        xtb = np.ascontiguousarray(x[b].T.astype(np.float32))  # [D, S]
        xt_bf = xtb.astype(bf)
        xqt_bf = np.ascontiguousarray(xtb[:, p::4]).astype(bf)  # [D, QL]
        t = np.arange(128)[:, None]
        l = np.arange(32)[None, :]
        mask = np.where(t <= 4 * l + p, 0.0, NEG).astype(np.float32)
        in_maps.append({
            "xt": xt_bf,
            "xqt": xqt_bf,
            "wa": wa_bf,
            "baq": baq,
            "bav": bav1,
            "wp": wp_bf,
            "bp": bp1,
            "mask": mask,
            "ones1": ones1,
        })
    return in_maps


def kernel(x, w_attn, b_attn, w_proj, b_proj):
    x = np.asarray(x, np.float32)
    w_attn = np.asarray(w_attn, np.float32)
    b_attn = np.asarray(b_attn, np.float32)
    w_proj = np.asarray(w_proj, np.float32)
    b_proj = np.asarray(b_proj, np.float32)

    if "nc" not in _CACHED:
        _CACHED["nc"] = build_nc()
    nc = _CACHED["nc"]
    in_maps = _host_inputs(x, w_attn, b_attn, w_proj, b_proj)
    res = run_bass_kernel_spmd(nc, in_maps, core_ids=list(range(8)))
    full = np.empty((B, S, D), np.float32)
    for c in range(8):
        b, p = c // 4, c % 4
        full[b, p::4] = res.results[c]["out"]
    return full


def run_traced(x, w_attn, b_attn, w_proj, b_proj):
    """Profiled run (test-only helper) — returns BassKernelResults."""
    if "nc" not in _CACHED:
        _CACHED["nc"] = build_nc()
    nc = _CACHED["nc"]
    in_maps = _host_inputs(
        np.asarray(x, np.float32), np.asarray(w_attn, np.float32),
        np.asarray(b_attn, np.float32), np.asarray(w_proj, np.float32),
        np.asarray(b_proj, np.float32),
    )
    return run_bass_kernel_spmd(
        nc, in_maps, core_ids=list(range(8)), trace=True, trace_cores=[0]
    )

